# revision 19
# baseline (speedup 1.0000x reference)
"""AutoCorrelation (Autoformer) Trainium2 kernel, 8-core data-parallel over batch.

Algorithm per core (one batch b):
  mean_value[b, tau] = (1/(H*E)) sum_c circular-crosscorr(q[:,c], k[:,c])[tau]
computed via a 16-subsequence DFT-128 decomposition (t = 16u + r):
  - stage A: per (r, c): DFT_128 of subsequence -> r-major packed spectra in
    SBUF (contiguous PSUM->SBUF copies spread over vector/scalar/gpsimd;
    stage-P slabs read the f-major order via 2-D strided APs).
  - stage P: cross-spectra batched 8 freqs/matmul; 16x16 diagonal blocks of
    each [128,128] PSUM tile hold P[a, b, f].  P_re/P_im accumulate in PSUM.
  - diag extraction: PSUM -> SBUF h2 -> one DMA per group into a DRAM arena,
    then 8 "skewed readback" DMAs pull the diagonal blocks straight into a
    zero-initialized SBUF tile laid out [f, a, u=66] with u = a+b+1 (+32 for
    the wrap window; wrap spill-over lands in a junk zone u 48:64), so a
    single grouped DVE reduce over `a` yields the HLO planes; twiddled
    IDFT-128 matmuls -> mean_value[b] (exact fp32).
  - AllReduce(sum) over 8 cores -> scores.  The trigger is issued as soon as
    mean_value lands; V^T is built during the collective window by xbar
    DMA-transposes straight from DRAM (no PE transposes, v never staged).
  - top-7 (vector.max/max_index) + softmax; out[t,c] = sum_i w_i *
    v[(t+delta_i) % L, c] via 7 dynamically-offset PE taps per 512-wide
    chunk (3 channel-chunks on PE, 1 on DVE scalar_tensor_tensor), then
    xbar DMA-transposes back to [t, c] and bf16 output stores.
"""

import os
import sys
import numpy as np
import ml_dtypes

for p in ("/opt/trn_rl_repo",):
    if p not in sys.path and os.path.isdir(p):
        sys.path.insert(0, p)

import concourse.bass as bass
import concourse.bacc as bacc
import concourse.tile as tile
import concourse.mybir as mybir
from concourse import bass_utils

F32 = mybir.dt.float32
F16 = mybir.dt.float16
BF16 = mybir.dt.bfloat16
U32 = mybir.dt.uint32
AL = mybir.AluOpType
AX = mybir.AxisListType
ACTF = mybir.ActivationFunctionType

B, L, H, E = 8, 2048, 8, 64
C = H * E            # 512
U, R = 128, 16       # L = R * U ; t = 16*u + r
NF = 65              # rfft freqs of DFT-128 we keep (0..64)
KW = 192             # K packed spectrum width (191 used)
NCORES = 8
TOPK = 7
W66 = 66             # DGS per-a block width (48 data + 16 junk + 2 pad)
FSTRIDE = 16 * 128 + 16  # diag-block (f) stride inside a pd plane arena


def _consts():
    u = np.arange(U)
    f = np.arange(NF)
    f2 = np.arange(1, 64)

    # Q-side DFT matrix, packed: cols 0..64 = cos, cols 65..127 = -sin (f=1..63)
    wq = np.zeros((U, 128), dtype=np.float16)
    wq[:, :NF] = np.cos(2 * np.pi * np.outer(u, f) / U)
    wq[:, NF:] = -np.sin(2 * np.pi * np.outer(u, f2) / U)

    # K-side adds +sin section (cols 128..190) = negated stored-im
    wk = np.zeros((U, KW), dtype=np.float16)
    wk[:, :128] = wq
    wk[:, 128:191] = np.sin(2 * np.pi * np.outer(u, f2) / U)

    # IDFT matrix on packed spectrum -> mean_value (incl 2x Hermitian weight and 1/(U*C))
    widft = np.zeros((128, U), dtype=np.float32)
    v = np.arange(U)
    scale = np.ones(NF); scale[1:64] = 2.0
    norm = 1.0 / (U * C)
    widft[:NF, :] = (scale[:, None] * np.cos(2 * np.pi * np.outer(f, v) / U)) * norm
    widft[NF:, :] = (-2.0 * np.sin(2 * np.pi * np.outer(f2, v) / U)) * norm

    # twiddles for the lo-diagonal terms, folded into extra IDFT slabs
    twv1 = np.zeros((128, 1), dtype=np.float32)
    twv2 = np.zeros((128, 1), dtype=np.float32)
    twv1[:NF, 0] = np.cos(2 * np.pi * f / U)
    twv2[:NF, 0] = -np.sin(2 * np.pi * f / U)
    twv1[NF:, 0] = np.cos(2 * np.pi * f2 / U)
    twv2[NF:, 0] = np.sin(2 * np.pi * f2 / U)
    widft_l1 = (twv1 * widft).astype(np.float32)
    widft_l2 = (twv2 * widft).astype(np.float32)

    ident = np.eye(128, dtype=np.float32)
    return wq, wk, widft, widft_l1, widft_l2, ident


def build_kernel(nc, no_collective=False, debug=False):
    q_ext = nc.dram_tensor("q", [L, C], F16, kind="ExternalInput")
    k_ext = nc.dram_tensor("k", [L, C], F16, kind="ExternalInput")
    v_ext = nc.dram_tensor("v", [L, C], BF16, kind="ExternalInput")
    wq_ext = nc.dram_tensor("wdftq", [U, 128], F16, kind="ExternalInput")
    wk_ext = nc.dram_tensor("wdftk", [U, KW], F16, kind="ExternalInput")
    widft_ext = nc.dram_tensor("widft", [128, U], F32, kind="ExternalInput")
    widftl1_ext = nc.dram_tensor("widftl1", [128, U], F32, kind="ExternalInput")
    widftl2_ext = nc.dram_tensor("widftl2", [128, U], F32, kind="ExternalInput")
    ident_ext = nc.dram_tensor("ident", [128, 128], F32, kind="ExternalInput")
    out_ext = nc.dram_tensor("out", [L, C], BF16, kind="ExternalOutput")

    NCH = C // 128  # 4 channel chunks
    # f-groups for batched cross-spectra (f=1..63); f=0/64 handled separately
    FGROUPS = [(1 + 8 * i, 8) for i in range(7)] + [(57, 7)]

    with tile.TileContext(nc) as tc:
        with (
            tc.tile_pool(name="const", bufs=1) as constp,
            tc.tile_pool(name="spec", bufs=1) as specp,
            tc.tile_pool(name="stage", bufs=2) as stagep,
            tc.tile_pool(name="vt", bufs=1) as vtp,
            tc.tile_pool(name="small", bufs=1) as smallp,
            tc.tile_pool(name="psa", bufs=3, space="PSUM") as psA,
            tc.tile_pool(name="psp", bufs=2, space="PSUM") as psP,
            tc.tile_pool(name="psm", bufs=1, space="PSUM") as psM,
            tc.tile_pool(name="dram", bufs=1, space="DRAM") as dramp,
        ):
            psmisc = psM.tile([128, 512], F32, tag="psmisc", bufs=1)

            # ---- PE clock warm-up while loads stream: HAM grants full clock
            # after ~3.4us of sustained activity; these dummies keep the PE
            # busy until stage A's first operands land.
            dumbf = constp.tile([128, 128], BF16, tag="dumbf")
            nc.vector.memset(dumbf[:], 0.0)
            for it in range(22):
                nc.tensor.matmul(
                    psmisc[:, 128:256], dumbf[:], dumbf[:], start=True, stop=True)

            # ---- input loads: q/k chunks interleaved across the two HWDGE
            # queues so each r-quarter (q AND k) lands as early as possible.
            xq = stagep.tile([128, R * C], F16, tag="xq", name="xq", bufs=1)
            xk = stagep.tile([128, R * C], F16, tag="xk", name="xk", bufs=1)
            xqv = xq.rearrange("u (r c) -> u r c", c=C)
            xkv = xk.rearrange("u (r c) -> u r c", c=C)
            qsrcv = q_ext.ap().rearrange("(u r) c -> u r c", r=R)
            ksrcv = k_ext.ap().rearrange("(u r) c -> u r c", r=R)

            wq_sb = constp.tile([U, 128], F16, tag="wq")
            wk_sb = constp.tile([U, KW], F16, tag="wk")
            nc.sync.dma_start(wq_sb[:], wq_ext.ap())
            nc.scalar.dma_start(wk_sb[:], wk_ext.ap())
            for j in range(8):
                qc = (xqv[:, j * 2:(j + 1) * 2, :], qsrcv[:, j * 2:(j + 1) * 2, :])
                kc = (xkv[:, j * 2:(j + 1) * 2, :], ksrcv[:, j * 2:(j + 1) * 2, :])
                if j % 2 == 0:
                    nc.sync.dma_start(*qc)
                    nc.scalar.dma_start(*kc)
                else:
                    nc.sync.dma_start(*kc)
                    nc.scalar.dma_start(*qc)

            # IDFT consts + identity on the gpsimd SWDGE queue (off HWDGE),
            # then the V load (bf16, consumed only in the collective window)
            widft_sb = constp.tile([128, U], F32, tag="widft")
            widftl1_sb = constp.tile([128, U], F32, tag="widftl1")
            widftl2_sb = constp.tile([128, U], F32, tag="widftl2")
            ident_sb = constp.tile([128, 128], F32, tag="ident")
            nc.gpsimd.dma_start(widft_sb[:], widft_ext.ap())
            nc.gpsimd.dma_start(widftl1_sb[:], widftl1_ext.ap())
            nc.gpsimd.dma_start(widftl2_sb[:], widftl2_ext.ap())
            nc.gpsimd.dma_start(ident_sb[:], ident_ext.ap())
            # V (bf16), behind q/k on the HWDGE queues; consumed only in the
            # collective window
            vstage = vtp.tile([128, R * C], BF16, tag="vstage", name="vstage")
            vstv = vstage.rearrange("p (j c) -> p j c", c=C)
            vsrcv = v_ext.ap().rearrange("(j p) c -> p j c", p=128)
            nc.sync.dma_start(vstv[:, 0:8, :], vsrcv[:, 0:8, :])
            nc.scalar.dma_start(vstv[:, 8:16, :], vsrcv[:, 8:16, :])

            ident_bf = constp.tile([128, 128], BF16, tag="identbf")
            nc.vector.tensor_copy(ident_bf[:], ident_sb[:])
            ones_row = constp.tile([1, 128], F32, tag="ones_row")
            nc.vector.memset(ones_row[:], 1.0)

            # preload the exp activation table off the critical path
            warm = smallp.tile([1, 1], F32, tag="warm")
            nc.vector.memset(warm[:], 0.0)
            nc.scalar.activation(warm[:], warm[:], ACTF.Exp)

            # skew-destination tile: [f-row, a, u] with u = a+b+1 (+32 for the
            # wrap window); cols u in 48:64 are the wrap-spill junk zone,
            # everything else must start zero.
            DGS = specp.tile([128, 16 * W66], F32, tag="dgs", name="dgs")
            nc.vector.memset(DGS[:], 0.0)

            # ---- stage A: subsequence spectra ----
            # f-major SBUF layout: col = fp*16 + r, so stage-P slabs over an
            # f-range are contiguous (matmul operands allow one free dim only).
            QS = [specp.tile([128, 128 * R], BF16, tag=f"qs{ch}", name=f"qs{ch}")
                  for ch in range(NCH)]
            KS = [specp.tile([128, KW * R], BF16, tag=f"ks{ch}", name=f"ks{ch}")
                  for ch in range(NCH)]

            QSvs = [QS[ch].rearrange("c (fp r) -> c fp r", r=R) for ch in range(NCH)]
            KSvs = [KS[ch].rearrange("c (fp r) -> c fp r", r=R) for ch in range(NCH)]
            for q4 in range(4):
                for ch in range(NCH):
                    ps = psA.tile([128, 512], F32, tag="psa", name="psq")
                    for rr in range(4):
                        r = 4 * q4 + rr
                        nc.tensor.matmul(
                            ps[:, rr * 128:(rr + 1) * 128],
                            xq[:, r * C + ch * 128:r * C + ch * 128 + 128],
                            wq_sb[:], start=True, stop=True)
                    src = ps.rearrange("c (r fp) -> c fp r", fp=128)
                    dst = QSvs[ch][:, :, q4 * 4:(q4 + 1) * 4]
                    if ch % 2 == 0:
                        nc.vector.tensor_copy(dst, src)
                    else:
                        nc.scalar.activation(dst, src, ACTF.Copy)
                for pi in (2 * q4, 2 * q4 + 1):
                    for ch in range(NCH):
                        ps = psA.tile([128, 512], F32, tag="psa", name="psk")
                        for rr in range(2):
                            r = 2 * pi + rr
                            slot = 1 - rr
                            nc.tensor.matmul(
                                ps[:, slot * 256:slot * 256 + KW],
                                xk[:, r * C + ch * 128:r * C + ch * 128 + 128],
                                wk_sb[:], start=True, stop=True)
                        # K stored r-reversed: block m holds subsequence 15-m
                        base = R - 2 - 2 * pi
                        src = ps.rearrange("c (s w) -> c w s", w=256)[:, 0:KW, :]
                        dst = KSvs[ch][:, :, base:base + 2]
                        if ch % 2 == 0:
                            nc.vector.tensor_copy(dst, src)
                        else:
                            nc.scalar.activation(dst, src, ACTF.Copy)

            # ---- stage P: cross spectra, 8 freqs per matmul ----
            def qslab(ch, f0, nf):
                return QS[ch][:, f0 * R:(f0 + nf) * R]

            def kslab(ch, f0, nf):
                return KS[ch][:, f0 * R:(f0 + nf) * R]

            # f = 0 and f = 64 (re only, P_im = 0 there)
            for bi, ff in enumerate((0, 64)):
                for ch in range(NCH):
                    nc.tensor.matmul(
                        psmisc[0:16, bi * 16:bi * 16 + 16],
                        qslab(ch, ff, 1), kslab(ch, ff, 1),
                        start=(ch == 0), stop=(ch == NCH - 1))
            h064 = stagep.tile([16, 32], F32, tag="h064", name="h064", bufs=1)
            nc.vector.tensor_copy(h064[:], psmisc[0:16, 0:32])
            pd064 = dramp.tile([512], F32, tag="pd064", name="pd064")
            nc.gpsimd.dma_start(
                pd064.rearrange("(p q) -> p q", q=32), h064[:])

            # two plane arenas with a uniform f-block stride (2064) across all
            # 63 freqs, so each skewed readback is a single 3-dim DMA
            pd_re = dramp.tile([8 * 8 * FSTRIDE], F32, tag="pdre", name="pdre")
            pd_im = dramp.tile([8 * 8 * FSTRIDE], F32, tag="pdim", name="pdim")
            for gi, (f0, nf) in enumerate(FGROUPS):
                n = nf * 16
                pre = psP.tile([128, 128], F32, tag="pre", name=f"pre{gi}")
                for ch in range(NCH):
                    nc.tensor.matmul(
                        pre[0:n, 0:n], qslab(ch, f0, nf), kslab(ch, f0, nf),
                        start=(ch == 0), stop=False)
                for ch in range(NCH):
                    nc.tensor.matmul(
                        pre[0:n, 0:n], qslab(ch, 64 + f0, nf),
                        kslab(ch, 64 + f0, nf),
                        start=False, stop=(ch == NCH - 1))
                pim = psP.tile([128, 128], F32, tag="pim", name=f"pim{gi}")
                for ch in range(NCH):
                    nc.tensor.matmul(
                        pim[0:n, 0:n], qslab(ch, 64 + f0, nf), kslab(ch, f0, nf),
                        start=(ch == 0), stop=False)
                for ch in range(NCH):
                    nc.tensor.matmul(
                        pim[0:n, 0:n], qslab(ch, f0, nf),
                        kslab(ch, 127 + f0, nf),
                        start=False, stop=(ch == NCH - 1))

                # PSUM -> SBUF (re cols 0:128, im 128:256) -> one DMA per plane
                # into the arenas (full rows; only diag blocks read back)
                h2 = stagep.tile([128, 256], F32, tag="phop", name=f"h2_{gi}", bufs=4)
                nc.vector.tensor_copy(h2[0:n, 0:128], pre[0:n, :])
                nc.scalar.activation(h2[0:n, 128:256], pim[0:n, :], ACTF.Copy)
                h1eng = (nc.gpsimd, nc.sync, nc.scalar)[gi % 3]
                h1eng.dma_start(
                    bass.AP(pd_re.tensor, gi * 8 * FSTRIDE, [[128, 128], [1, 128]]),
                    h2[:, 0:128])
                h1eng2 = (nc.sync, nc.scalar, nc.gpsimd)[gi % 3]
                h1eng2.dma_start(
                    bass.AP(pd_im.tensor, gi * 8 * FSTRIDE, [[128, 128], [1, 128]]),
                    h2[:, 128:256])

            # ---- skewed readbacks: pd diag blocks -> DGS[f, 67a + b + off]
            # f-rows (0:65): winA(off 1) <- re plane, winB(off 33) <- im plane
            # f2-rows (65:128): winA <- im plane, winB <- re plane
            dgs_ap = DGS[:]
            dgs_ps = dgs_ap.ap[0][0]
            dgs_off = dgs_ap.offset

            for pi_, (row0, woff) in enumerate((
                    (1, 1),        # f-rows winA  <- re plane
                    (65, 1),       # f2-rows winA <- im plane
                    (1, 33),       # f-rows winB  <- im plane
                    (65, 33))):    # f2-rows winB <- re plane
                arena = (pd_re, pd_im, pd_im, pd_re)[pi_]
                dst = bass.AP(dgs_ap.tensor, dgs_off + row0 * dgs_ps + woff,
                              [[dgs_ps, 63], [67, 16], [1, 16]])
                src = bass.AP(arena.tensor, 0, [[FSTRIDE, 63], [128, 16], [1, 16]])
                (nc.sync, nc.scalar)[pi_ % 2].dma_start(dst, src)
            # f = 0 / 64 (re plane only; im contributions are zero)
            for bi in range(2):
                dst = bass.AP(dgs_ap.tensor, dgs_off + bi * 64 * dgs_ps + 1,
                              [[dgs_ps, 1], [67, 16], [1, 16]])
                src = bass.AP(pd064.tensor, bi * 16, [[32, 16], [1, 16]])
                (nc.sync, nc.scalar)[bi].dma_start(dst, src)

            # keep the PE HAM-warm across the readback/reduce gap
            for it in range(25):
                nc.tensor.matmul(
                    psmisc[:, 128:512], dumbf[:], QS[0][:, 0:384],
                    start=True, stop=True)

            # ---- single grouped reduce over a + IDFT -> mean_value ----
            HLO = smallp.tile([128, 48], F32, tag="hlo")
            nc.vector.tensor_reduce(
                HLO[:], DGS.rearrange("p (a u) -> p u a", u=W66)[:, 0:48, :],
                axis=AX.X, op=AL.add)
            LO1, HI, LO2 = HLO[:, 0:16], HLO[:, 16:32], HLO[:, 32:48]

            mvreg = psmisc[:, 32:48]
            nc.tensor.matmul(mvreg, widft_sb[:], HI, start=True, stop=False)
            nc.tensor.matmul(mvreg, widftl1_sb[:], LO1, start=False, stop=False)
            nc.tensor.matmul(mvreg, widftl2_sb[:], LO2, start=False, stop=True)
            mv_sb = smallp.tile([128, R], F32, tag="mv")
            nc.vector.tensor_copy(mv_sb[:], mvreg)

            mv_dram = dramp.tile([L], F32, tag="mvd")
            ag_dram = dramp.tile([NCORES * L], F32, tag="agd")
            sc_dram = dramp.tile([L], F32, tag="scd")
            nc.sync.dma_start(mv_dram.rearrange("(p w) -> p w", w=R), mv_sb[:])

            # ---- gather per-core mean_values (AllGather has a much lower
            # latency floor than AllReduce); the batch-sum happens on DVE ----
            if no_collective:
                for cc in range(NCORES):
                    nc.gpsimd.dma_start(
                        ag_dram[cc * L:(cc + 1) * L], mv_dram[:])
            else:
                nc.gpsimd.collective_compute(
                    "AllGather",
                    AL.bypass,
                    replica_groups=[list(range(NCORES))],
                    ins=[mv_dram.opt()],
                    outs=[ag_dram.opt()],
                )

            # ---- fill the collective window: V^T via PE transposes (also
            # keeps the PE HAM-warm through the collective; xbar transposes
            # may not run concurrently with collectives) ----
            VT = [vtp.tile([128, 2 * L], BF16, tag=f"vt{ch}", name=f"vt{ch}")
                  for ch in range(NCH)]
            for j in range(R):
                for ch in range(NCH):
                    pst = psA.tile([128, 128], BF16, tag="psa", name="pst")
                    nc.tensor.transpose(
                        pst[:], vstage[:, j * C + ch * 128:j * C + ch * 128 + 128],
                        ident_bf[:])
                    eng = (nc.vector.tensor_copy if (j + ch) % 2 == 0 else
                           lambda d, s: nc.scalar.activation(d, s, ACTF.Copy))
                    eng(VT[ch][:, j * 128:(j + 1) * 128], pst[:])
            for ch in range(NCH):
                if ch % 2 == 0:
                    nc.vector.tensor_copy(VT[ch][:, L:2 * L], VT[ch][:, 0:L])
                else:
                    nc.scalar.activation(VT[ch][:, L:2 * L], VT[ch][:, 0:L],
                                         ACTF.Copy)

            mvl_sb = smallp.tile([1, L], F32, tag="mvl")
            nc.scalar.dma_start(mvl_sb[:], mv_dram.rearrange("(o l) -> o l", o=1))

            # batch-sum of the gathered mean_values -> scores
            agt = smallp.tile([128, 16 * NCORES], F32, tag="agt")
            agtv = agt.rearrange("p (w a) -> p w a", a=NCORES)
            nc.sync.dma_start(
                agtv, bass.AP(ag_dram.tensor, 0, [[16, 128], [1, 16], [L, NCORES]]))
            sc128 = smallp.tile([128, R], F32, tag="sc128")
            nc.vector.tensor_reduce(sc128[:], agtv, axis=AX.X, op=AL.add)
            nc.sync.dma_start(sc_dram.rearrange("(p w) -> p w", w=R), sc128[:])
            sc_sb = smallp.tile([1, L], F32, tag="scsb")
            nc.sync.dma_start(sc_sb[:], sc_dram.rearrange("(o l) -> o l", o=1))

            # free-running dummies bridge the gap between the V transposes
            # and the collective completing
            for it in range(95):
                nc.tensor.matmul(
                    psmisc[:, 128:512], dumbf[:], QS[0][:, 0:384],
                    start=True, stop=True)
            # PE re-warm gated on the collective result (K=1 outer products on
            # sc_sb): covers the HAM window through the topk chain so the
            # taps start at full clock.
            for it in range(12):
                nc.tensor.matmul(
                    psmisc[:, 256:384], sc_sb[0:1, 0:128],
                    sc_sb[0:1, 0:128], start=True, stop=True)

            # ---- top-7 + softmax weights ----
            mx8 = smallp.tile([1, 8], F32, tag="mx8")
            idx8 = smallp.tile([1, 8], U32, tag="idx8")
            nc.vector.max(mx8[:], sc_sb[:])
            nc.vector.max_index(idx8[:], mx8[:], sc_sb[:])

            _, deltas = nc.values_load_multi_w_load_instructions(
                idx8[0:1, 0:TOPK], min_val=0, max_val=L - 1,
                skip_runtime_bounds_check=True,
                engines=(mybir.EngineType.PE, mybir.EngineType.DVE))

            wv = smallp.tile([1, 8], F32, tag="wv")
            nc.vector.memset(wv[:], 0.0)
            for i in range(TOPK):
                nc.vector.tensor_copy(
                    wv[0:1, i:i + 1], mvl_sb[0:1, bass.ds(deltas[i], 1)])
            nc.scalar.activation(wv[0:1, 0:TOPK], wv[0:1, 0:TOPK], ACTF.Exp)
            wsum = smallp.tile([1, 1], F32, tag="wsum")
            nc.vector.reduce_sum(wsum[:], wv[0:1, 0:TOPK], axis=AX.X)
            wrec = smallp.tile([1, 1], F32, tag="wrec")
            nc.vector.reciprocal(wrec[:], wsum[:])
            nc.vector.tensor_scalar(
                wv[0:1, 0:TOPK], wv[0:1, 0:TOPK], wrec[:], None, AL.mult)

            # broadcast weights across partitions via a rank-1 PE matmul
            wbreg = psmisc[:, 48:56]
            nc.tensor.matmul(wbreg, ones_row[:], wv[:], start=True, stop=True)
            wb_sb = smallp.tile([128, 8], F32, tag="wb")
            nc.vector.tensor_copy(wb_sb[:], wbreg)

            # ---- 7-tap weighted shifted sum ----
            # reference: rolled[l] = v[(l - delta) % L] -> doubled-V offset L - delta
            offs = [L - d for d in deltas]
            WIall = constp.tile([128, TOPK * 128], BF16, tag="wiall")
            for i in range(TOPK):
                nc.vector.tensor_scalar(
                    WIall[:, i * 128:(i + 1) * 128], ident_bf[:],
                    wb_sb[:, i:i + 1], None, AL.mult)
            WI = [WIall[:, i * 128:(i + 1) * 128] for i in range(TOPK)]
            ACC = [specp.tile([128, L], BF16, tag=f"qs{ch}", name=f"acc{ch}")
                   for ch in range(NCH)]
            ot = stagep.tile([128, R * C], BF16, tag="xq", name="ot", bufs=1)
            otv = ot.rearrange("p (j c) -> p j c", c=C)
            outv = out_ext.ap().rearrange("(j p) c -> j p c", p=128)

            for ks in range(4):
                for ch in range(NCH):
                    pt = psA.tile([128, 512], F32, tag="psa", name="pt")
                    for i in range(TOPK):
                        nc.tensor.matmul(
                            pt[:], WI[i][:],
                            VT[ch][:, ks * 512:][:, bass.ds(offs[i], 512)],
                            start=(i == 0), stop=(i == TOPK - 1))
                    dst = ACC[ch][:, ks * 512:(ks + 1) * 512]
                    if ch % 2 == 0:
                        nc.vector.tensor_copy(dst, pt[:])
                    else:
                        nc.scalar.activation(dst, pt[:], ACTF.Copy)
                # xbar transpose back to [t, c] + output stores
                for ch in range(NCH):
                    eng = (nc.sync, nc.scalar)[ch % 2]
                    eng.dma_start_transpose(
                        otv[:, ks * 4:(ks + 1) * 4, ch * 128:(ch + 1) * 128],
                        ACC[ch][:, ks * 512:(ks + 1) * 512])
                for jj in range(4 * ks, 4 * ks + 4):
                    eng = (nc.sync, nc.scalar)[jj % 2]
                    eng.dma_start(outv[jj], otv[:, jj, :])

            if debug:
                dbg_mv = nc.dram_tensor("dbg_mv", [L], F32, kind="ExternalOutput")
                dbg_sc = nc.dram_tensor("dbg_sc", [L], F32, kind="ExternalOutput")
                dbg_hlo = nc.dram_tensor("dbg_hlo", [128, 48], F32, kind="ExternalOutput")
                dbg_wv = nc.dram_tensor("dbg_wv", [1, 8], F32, kind="ExternalOutput")
                dbg_idx = nc.dram_tensor("dbg_idx", [1, 8], U32, kind="ExternalOutput")
                dbg_vt = nc.dram_tensor("dbg_vt", [128, 2 * L], BF16, kind="ExternalOutput")
                dbg_acc = nc.dram_tensor("dbg_acc", [128, L], BF16, kind="ExternalOutput")
                dbg_dgs = nc.dram_tensor("dbg_dgs", [128, 16 * W66], F32, kind="ExternalOutput")
                nc.gpsimd.dma_start(dbg_mv.ap().rearrange("(o l) -> o l", o=1), mvl_sb[:])
                nc.gpsimd.dma_start(dbg_sc.ap().rearrange("(o l) -> o l", o=1), sc_sb[:])
                nc.gpsimd.dma_start(dbg_hlo.ap(), HLO[:])
                nc.gpsimd.dma_start(dbg_wv.ap(), wv[:])
                nc.gpsimd.dma_start(dbg_idx.ap(), idx8[:])
                nc.gpsimd.dma_start(dbg_vt.ap(), VT[0][:])
                nc.gpsimd.dma_start(dbg_acc.ap(), ACC[0][:])
                nc.gpsimd.dma_start(dbg_dgs.ap(), DGS[:])

            # ---- dead-code sinks for the warm-up matmuls ----
            wsink = dramp.tile([4], F32, tag="wsink", name="wsink")
            wsb = smallp.tile([1, 2], F32, tag="wsb")
            nc.vector.tensor_copy(wsb[0:1, 0:1], psmisc[0:1, 128:129])
            nc.vector.tensor_copy(wsb[0:1, 1:2], psmisc[0:1, 256:257])
            nc.sync.dma_start(wsink.rearrange("(o x) -> o x", o=1)[:, 0:2], wsb[:])

    return nc


_NC_CACHE = {}


def _get_nc():
    if "nc" not in _NC_CACHE:
        nc = bacc.Bacc(
            "TRN2", target_bir_lowering=False, debug=False, num_devices=NCORES)
        build_kernel(nc)
        nc.compile()
        _NC_CACHE["nc"] = nc
    return _NC_CACHE["nc"]


def _in_maps(queries, keys, values):
    wq, wk, widft, wl1, wl2, ident = _consts()
    maps = []
    for b in range(B):
        maps.append({
            "q": np.ascontiguousarray(queries[b].reshape(L, C), dtype=np.float16),
            "k": np.ascontiguousarray(keys[b].reshape(L, C), dtype=np.float16),
            "v": np.ascontiguousarray(values[b].reshape(L, C)).astype(
                ml_dtypes.bfloat16),
            "wdftq": wq, "wdftk": wk,
            "widft": widft, "widftl1": wl1, "widftl2": wl2,
            "ident": ident,
        })
    return maps


def run(queries, keys, values, trace=False):
    nc = _get_nc()
    res = bass_utils.run_bass_kernel_spmd(
        nc, _in_maps(queries, keys, values),
        core_ids=list(range(NCORES)), trace=trace)
    outs = [np.asarray(res.results[b]["out"]).astype(np.float32).reshape(L, H, E)
            for b in range(B)]
    return np.stack(outs, axis=0), res


def kernel(queries, keys, values, attn_mask=None):
    out, _ = run(np.asarray(queries), np.asarray(keys), np.asarray(values))
    return out.astype(np.float32)
